# revision 11
# baseline (speedup 1.0000x reference)
"""Trainium2 Bass kernel for Luong-style attention.

Reference computation (per full problem):
    h = decoder_hidden @ W.T + b          # [B, De]
    enc = encoder_output.transpose(1,0,2) # [B, S, De]
    a = softmax(einsum('bsd,bd->bs', enc, h), axis=1)
    context = einsum('bs,bsd->bd', a, enc)  # [B, De]

Shapes: B=64, S=4096, Dd=1024, De=512 (f32).

Strategy: data-parallel over B across 8 NeuronCores (B_local=8 each).
encoder_output is the huge tensor (512 MB); each core streams its
64 MB f32 shard from HBM exactly once.  v2 design:
  - the streaming DMA casts f32->fp16 inline (SWDGE / gpsimd DMA),
    so SBUF holds only 1 MB fp16 tiles and the Scalar engine no
    longer burns ~120 us on casts,
  - scores via DVE scalar_tensor_tensor (product + row-sum fused)
    against a partition-broadcast fp16 copy of h,
  - online (flash-attention style) softmax: running max/sum and a
    running fp32 context accumulator rescaled per chunk, so there is
    no end-of-kernel combine pass over all chunks,
  - context accumulated TRANSPOSED in one PSUM bank per chunk
    (lhsT = fp16 enc d-slice as weights with fast-weight-load,
    rhs = fp16 prob column, N=1; accumulation group opened by a
    bank-wide zero matmul since start=True clears has_written
    bank-wide),
  - W and decoder_hidden are passed in pre-transposed from the host
    (pure layout prep) so setup needs no PE transposes,
  - uneven chunk schedule (the last chunks are short) to shrink the
    compute-only drain tail after the final DMA.
No collectives needed.
"""

import numpy as np

import concourse.bass as bass
import concourse.bacc as bacc_mod
import concourse.tile as tile
import concourse.mybir as mybir
from concourse import masks
from concourse.bass_utils import run_bass_kernel_spmd

F32 = mybir.dt.float32
F16 = mybir.dt.float16
ALU = mybir.AluOpType
ACTF = mybir.ActivationFunctionType
AX = mybir.AxisListType

NCORES = 8
B = 8          # per-core batch
S = 4096
DD = 1024
DE = 512
P = 128        # s-values per tile
M = DE // P              # 4 d-chunks
NTILES = S // P          # 32
CHUNKS = [4, 4, 4, 4, 4, 4, 4, 2, 1, 1]   # tiles per softmax chunk
assert sum(CHUNKS) == NTILES
CMAX = max(CHUNKS)

# "swdge_cast": stream enc with inline f32->f16 cast on the gpsimd DMA path.
#   (measured: the single SWDGE context serializes ~2us of completion latency
#   per DMA -> stream runs at ~280 GB/s. kept only for reference)
# "hwdge": plain f32 HWDGE stream (sustains ~360 GB/s); scores read the f32
#   tile directly on DVE, ACT casts to fp16 for the context-matmul path.
DMA_MODE = "hwdge"


def build_nc():
    nchunk = len(CHUNKS)
    nc = bacc_mod.Bacc("TRN2", target_bir_lowering=False, debug=False)
    decT_d = nc.dram_tensor("decT", [DD, B], F32, kind="ExternalInput")
    enc_d = nc.dram_tensor("encoder_output", [S, B, DE], F32, kind="ExternalInput")
    wt_d = nc.dram_tensor("WT", [DD, DE], F32, kind="ExternalInput")
    b_d = nc.dram_tensor("b", [DE], F32, kind="ExternalInput")
    out_d = nc.dram_tensor("out", [B, DE], F32, kind="ExternalOutput")

    with tile.TileContext(nc) as tc:
        with (
            tc.tile_pool(name="const", bufs=1) as const_pool,
            tc.tile_pool(name="persist", bufs=1) as persist_pool,
            tc.tile_pool(name="enc16", bufs=8) as enc16_pool,
            tc.tile_pool(name="enc32", bufs=5) as enc32_pool,
            tc.tile_pool(name="scratch", bufs=4) as scratch_pool,
            tc.tile_pool(name="sct", bufs=12) as sct_pool,
            tc.tile_pool(name="probs", bufs=3) as p_pool,
            tc.tile_pool(name="pt", bufs=6) as pt_pool,
            tc.tile_pool(name="stat", bufs=6) as stat_pool,
        ):
            # ---------------- constants ----------------
            ident = const_pool.tile([P, P], F32)
            masks.make_identity(nc, ident[:])
            ones_f32 = const_pool.tile([1, P], F32)
            nc.vector.memset(ones_f32[:], 1.0)
            ones16 = const_pool.tile([1, P], F16)
            nc.vector.memset(ones16[:], 1.0)
            zrow16 = const_pool.tile([1, M * B], F16)
            nc.vector.memset(zrow16[:], 0.0)
            # row-broadcast selectors: sel[:, bb, :] is [8, 128] with row bb
            # all-ones; matmul(sel_bb, x) broadcasts x's row bb to all
            # 128 partitions without any cross-partition DMA.
            sel = const_pool.tile([B, B, P], F32)
            nc.gpsimd.memset(sel[:], 0.0)
            nc.gpsimd.affine_select(
                out=sel[:], in_=sel[:],
                compare_op=ALU.not_equal, fill=1.0, base=0,
                pattern=[[-1, B], [0, P]], channel_multiplier=1)

            # ---------------- persistent state ----------------
            # hb matches the dtype the score STT reads (f32 tiles in hwdge
            # mode, f16 tiles in swdge_cast mode)
            hb = persist_pool.tile([P, B, DE],
                                   F16 if DMA_MODE == "swdge_cast" else F32)
            # ping-pong online-softmax state
            Mr_t = persist_pool.tile([B, 2], F32)
            lr_t = persist_pool.tile([B, 2], F32)
            acc_t = [persist_pool.tile([P, M, B], F32, name=f"acc{i}")
                     for i in range(2)]
            nc.vector.memset(Mr_t[:], -1e30)
            nc.vector.memset(lr_t[:], 0.0)
            nc.vector.memset(acc_t[0][:], 0.0)

            # ---------------- setup: h = dec @ W.T + b ----------------
            setup_cm = tc.tile_pool(name="setup_sb", bufs=1)
            setup_sb = setup_cm.__enter__()
            psum_setup_cm = tc.tile_pool(name="psum_setup", bufs=2, space="PSUM")
            psum_setup = psum_setup_cm.__enter__()

            decT_sb = setup_sb.tile([P, DD // P, B], F32)
            wt_sb = setup_sb.tile([P, DD // P, DE], F32)
            bias_sb = setup_sb.tile([1, DE], F32)
            h_sb = setup_sb.tile([B, DE], F32)
            # issue setup loads on the Activation HWDGE queue so the enc
            # stream (SP queue) starts from t=0
            for kc in range(DD // P):
                nc.scalar.dma_start(decT_sb[:, kc, :], decT_d[kc * P:(kc + 1) * P, :])
                nc.scalar.dma_start(wt_sb[:, kc, :], wt_d[kc * P:(kc + 1) * P, :])
            nc.scalar.dma_start(bias_sb[:], b_d[None, :])

            h_ps = psum_setup.tile([B, DE], F32, tag="su")
            for kc in range(DD // P):
                nc.tensor.matmul(h_ps[:], decT_sb[:, kc, :], wt_sb[:, kc, :],
                                 start=(kc == 0), stop=False)
            nc.tensor.matmul(h_ps[:], ones_f32[0:1, 0:B], bias_sb[:],
                             start=False, stop=True)
            nc.vector.tensor_copy(h_sb[:], h_ps[:])

            # broadcast h along partitions into fp16 hb via selector matmuls
            for bb in range(B):
                hp = psum_setup.tile([P, DE], F32, tag="su")
                nc.tensor.matmul(hp[:], sel[:, bb, :], h_sb[:],
                                 start=True, stop=True)
                nc.vector.tensor_copy(hb[:, bb, :], hp[:])

            psum_setup_cm.__exit__(None, None, None)
            setup_cm.__exit__(None, None, None)

            # ---------------- main loop PSUM pools ----------------
            sc_cm = tc.tile_pool(name="psum_sc", bufs=3, space="PSUM")
            psum_sc = sc_cm.__enter__()
            tr_cm = tc.tile_pool(name="psum_tr", bufs=3, space="PSUM")
            psum_tr = tr_cm.__enter__()
            ctx_cm = tc.tile_pool(name="psum_ctx", bufs=2, space="PSUM")
            psum_ctx = ctx_cm.__enter__()

            jglobal = 0
            for ci, ct in enumerate(CHUNKS):
                cur, nxt = ci % 2, (ci + 1) % 2
                scT = psum_sc.tile([B, CMAX * P], F32)
                ets = []
                for t in range(ct):
                    j = jglobal + t
                    et = enc16_pool.tile([P, B, DE], F16)
                    if DMA_MODE == "swdge_cast":
                        nc.gpsimd.dma_start(et[:], enc_d[j * P:(j + 1) * P, :, :])
                        score_src = et
                    else:
                        et32 = enc32_pool.tile([P, B, DE], F32)
                        nc.sync.dma_start(et32[:], enc_d[j * P:(j + 1) * P, :, :])
                        nc.scalar.copy(et[:], et32[:])
                        score_src = et32
                    ets.append(et)
                    sct = sct_pool.tile([P, B], F32, tag="sct")
                    for bb in range(B):
                        junk = scratch_pool.tile([P, DE], F16, tag="junk")
                        nc.vector.scalar_tensor_tensor(
                            out=junk[:],
                            in0=score_src[:, bb, :],
                            scalar=1.0,
                            in1=hb[:, bb, :],
                            op0=ALU.mult,
                            op1=ALU.mult,
                            accum_out=sct[:, bb:bb + 1],
                        )
                    nc.tensor.transpose(scT[:, t * P:(t + 1) * P], sct[:], ident[:])
                jglobal += ct

                # ---- online softmax stats ----
                m_c = stat_pool.tile([B, 1], F32, tag="mc")
                nc.vector.reduce_max(m_c[:], scT[:, :ct * P], axis=AX.X)
                # Mnew = max(Mr, m_c)  (stored into ping-pong slot nxt)
                nc.vector.tensor_tensor(
                    out=Mr_t[:, nxt:nxt + 1], in0=Mr_t[:, cur:cur + 1],
                    in1=m_c[:], op=ALU.max)
                negM = stat_pool.tile([B, 1], F32, tag="negm")
                nc.vector.tensor_scalar_mul(negM[:], Mr_t[:, nxt:nxt + 1], -1.0)
                # alpha = exp(Mr_old - Mnew)
                alpha = stat_pool.tile([B, 1], F32, tag="alpha")
                nc.scalar.activation(alpha[:], Mr_t[:, cur:cur + 1], ACTF.Exp,
                                     bias=negM[:], scale=1.0)
                # p = exp(scores - Mnew), l_c = row-sum
                l_c = stat_pool.tile([B, 1], F32, tag="lc")
                p_sb = p_pool.tile([B, CMAX * P], F32)
                nc.scalar.activation(p_sb[:, :ct * P], scT[:, :ct * P], ACTF.Exp,
                                     bias=negM[:], scale=1.0, accum_out=l_c[:])
                # lr_new = lr_old * alpha + l_c
                nc.vector.scalar_tensor_tensor(
                    out=lr_t[:, nxt:nxt + 1], in0=lr_t[:, cur:cur + 1],
                    scalar=alpha[:, 0:1],
                    in1=l_c[:], op0=ALU.mult, op1=ALU.add)

                # alpha broadcast to [P, M*B]: transpose to [1, B], replicate to
                # [1, M*B], then a K=1 matmul against an all-ones column.
                aT_ps = psum_tr.tile([B, P], F32, tag="tr")
                nc.tensor.transpose(aT_ps[0:1, 0:B], alpha[:, 0:1], ident[0:B, 0:B])
                aT = stat_pool.tile([1, M * B], F32, tag="aT")
                for mm in range(M):
                    nc.vector.tensor_copy(aT[:, mm * B:(mm + 1) * B], aT_ps[0:1, 0:B])
                ab_ps = psum_tr.tile([P, M * B], F32, tag="tr")
                nc.tensor.matmul(ab_ps[:], ones_f32[0:1, :], aT[:],
                                 start=True, stop=True)
                alpha_bc = stat_pool.tile([P, M * B], F32, tag="abc")
                nc.vector.tensor_copy(alpha_bc[:], ab_ps[:])

                # ---- transposed context partial for this chunk, one PSUM bank ----
                ctx_ps = psum_ctx.tile([P, M, B], F32)
                nc.tensor.matmul(ctx_ps[:], ones16[:], zrow16[:],
                                 start=True, stop=False)
                for t in range(ct):
                    ptp = psum_tr.tile([P, B], F32, tag="tr")
                    nc.tensor.transpose(ptp[:], p_sb[:, t * P:(t + 1) * P],
                                        ident[0:B, 0:B])
                    pts = pt_pool.tile([P, B], F16)
                    nc.scalar.copy(pts[:], ptp[:])
                    for bb in range(B):
                        for mm in range(M):
                            last = (t == ct - 1 and bb == B - 1 and mm == M - 1)
                            nc.tensor.matmul(
                                ctx_ps[:, mm, bb:bb + 1],
                                ets[t][:, bb, mm * P:(mm + 1) * P],
                                pts[:, bb:bb + 1],
                                start=False, stop=last)

                # ---- acc_new = acc_old * alpha + ctx_chunk ----
                tmp = stat_pool.tile([P, M * B], F32, tag="tmp")
                nc.vector.tensor_tensor(
                    out=tmp[:],
                    in0=acc_t[cur][:].rearrange("p m b -> p (m b)"),
                    in1=alpha_bc[:], op=ALU.mult)
                nc.vector.tensor_tensor(
                    out=acc_t[nxt][:].rearrange("p m b -> p (m b)"),
                    in0=tmp[:],
                    in1=ctx_ps[:].rearrange("p m b -> p (m b)"), op=ALU.add)

            # ---------------- finalize: out = acc.T / l ----------------
            fin = len(CHUNKS) % 2
            rl = stat_pool.tile([B, 1], F32, tag="rl")
            nc.vector.reciprocal(rl[:], lr_t[:, fin:fin + 1])
            out_sb = persist_pool.tile([B, DE], F32)
            for mm in range(M):
                op_ps = psum_tr.tile([B, P], F32, tag="tr")
                nc.tensor.transpose(op_ps[:], acc_t[fin][:, mm, :], ident[:])
                nc.vector.tensor_scalar(
                    out=out_sb[:, mm * P:(mm + 1) * P], in0=op_ps[:],
                    scalar1=rl[:, 0:1], scalar2=None, op0=ALU.mult)
            nc.sync.dma_start(out_d[:], out_sb[:])

            ctx_cm.__exit__(None, None, None)
            tr_cm.__exit__(None, None, None)
            sc_cm.__exit__(None, None, None)

    nc.compile()
    if not nc.is_finalized():
        nc.finalize()
    return nc


_NC = None


def make_in_maps(decoder_hidden, encoder_output, W, b):
    decoder_hidden = np.ascontiguousarray(decoder_hidden, dtype=np.float32)
    encoder_output = np.ascontiguousarray(encoder_output, dtype=np.float32)
    WT = np.ascontiguousarray(np.asarray(W, dtype=np.float32).T)
    b = np.ascontiguousarray(b, dtype=np.float32)
    in_maps = []
    for i in range(NCORES):
        sl = slice(i * B, (i + 1) * B)
        in_maps.append({
            "decT": np.ascontiguousarray(decoder_hidden[sl].T),
            "encoder_output": np.ascontiguousarray(encoder_output[:, sl, :]),
            "WT": WT,
            "b": b,
        })
    return in_maps


def kernel(decoder_hidden, encoder_output, W, b):
    global _NC
    if _NC is None:
        _NC = build_nc()
    in_maps = make_in_maps(decoder_hidden, encoder_output, W, b)
    res = run_bass_kernel_spmd(_NC, in_maps, core_ids=list(range(NCORES)))
    return np.concatenate([res.results[i]["out"] for i in range(NCORES)], axis=0)
